# revision 6
# baseline (speedup 1.0000x reference)
"""GNN message passing (GraphConv_CA) kernel for Trainium2 (8 NeuronCores).

Problem: embed [50000, 64] f32; edge_index [2, 800000] i64; trend [800000] f32.
Per hop (x3): msg = agg[row] * trend; agg = segment_sum(msg, col, N).
Output: [50000, 4, 64] = concat(embed, hop1, hop2, hop3) along axis 1.

Strategy (gather-DMA + one-hot-matmul scatter):
  - Destination columns are grouped into 128-col tiles; tiles are assigned
    to (core, slot) pairs, load-balanced so each core gets an equal number
    of edge chunks. Core c computes the [128, 64] aggregation of each of
    its slots in PSUM and writes a [128, TSLOTS*64] slice; an AllGather
    rebuilds the full replicated agg for the next hop.
  - Node storage layout is permuted: node g (tile t = g//128, p = g%128,
    assigned to core/slot (c, s)) lives at row c*128*TSLOTS + p*TSLOTS + s
    of the [SRC_ROWS, 64] agg tensor, which is exactly the AllGather
    concatenation of the per-core [128, TSLOTS, 64] slices. The host
    permutes embed into this layout up front so all hops share one set of
    gather indices.
  - Per hop per core: dma_gather edge source rows (two instructions per
    block: gather indices are int16, so rows >= 32768 use a shifted view),
    DVE-multiply by per-edge trend, then per slot build one-hot matrices
    B[e, j] = (col_in_tile[e] == j) on DVE and accumulate
    psum += B.T @ gathered via the tensor engine (exact, race-free).
  - dma_scatter_add is NOT used: its CCE read-modify-write races on
    duplicate destination indices (verified on hardware).
"""

import sys

sys.path.insert(0, "/opt/trn_rl_repo")

import numpy as np

import concourse.bacc as bacc
import concourse.mybir as mybir
import concourse.tile as tile
from concourse.bass_utils import run_bass_kernel_spmd

F32 = mybir.dt.float32
I16 = mybir.dt.int16


class Cfg:
    def __init__(self, N=50000, E=800000, D=64, HOPS=3, NCORES=8,
                 ROW_SPLIT=32768, CHUNK_CAP=88):
        self.N, self.E, self.D, self.HOPS, self.NCORES = N, E, D, HOPS, NCORES
        self.CHUNK_CAP = CHUNK_CAP
        self.TILES_G = -(-N // 128)                 # global col tiles
        self.TSLOTS = -(-self.TILES_G // NCORES)    # slots per core
        self.BPR = 128 * self.TSLOTS                # agg rows per core block
        self.SRC_ROWS = NCORES * self.BPR
        self.RS2 = min(ROW_SPLIT, self.SRC_ROWS)
        self.HI2 = self.SRC_ROWS - self.RS2
        # filled by preprocess:
        self.KLO = None      # [TSLOTS] lo-chunk capacity per slot
        self.KHI = None
        self.blocks = None   # list of lists of slot ids
        self.assign = None   # [NCORES, TSLOTS] -> global tile id (or -1)
        self.KMAX = None


def _wrap16(arr):
    """[L] int16 -> [128, L//16]: position i at (i%16, i//16), replicated
    to 8 groups of 16 partitions."""
    w = arr.reshape(-1, 16).T
    return np.tile(w, (8, 1)).copy()


def preprocess(embed, edge_index, trend, cfg: Cfg):
    N, D, NC, TS = cfg.N, cfg.D, cfg.NCORES, cfg.TSLOTS
    row = np.asarray(edge_index[0], dtype=np.int64)
    col = np.asarray(edge_index[1], dtype=np.int64)
    trend = np.asarray(trend, dtype=np.float32)

    tile_of = col >> 7
    colloc = (col & 127).astype(np.float32)

    # per-tile edge lists
    order = np.argsort(tile_of, kind="stable")
    tsorted = tile_of[order]
    starts = np.searchsorted(tsorted, np.arange(cfg.TILES_G + 1))
    cnt = np.diff(starts)

    # assignment: sort tiles by total chunk count desc; rank groups of NC
    ktot = -(-cnt // 128)
    tile_rank = np.argsort(-ktot, kind="stable")
    assign = -np.ones((NC, TS), dtype=np.int64)
    for s in range(TS):
        grp = tile_rank[s * NC:(s + 1) * NC]
        for c, t in enumerate(grp):
            assign[c, s] = t
    cfg.assign = assign

    # srcidx of every node under the permuted layout
    core_of_tile = np.zeros(cfg.TILES_G, dtype=np.int64)
    slot_of_tile = np.zeros(cfg.TILES_G, dtype=np.int64)
    for c in range(NC):
        for s in range(TS):
            t = assign[c, s]
            if t >= 0 and t < cfg.TILES_G:
                core_of_tile[t] = c
                slot_of_tile[t] = s
    nodes = np.arange(N, dtype=np.int64)
    srcidx_of_node = (core_of_tile[nodes >> 7] * cfg.BPR
                      + (nodes & 127) * TS + slot_of_tile[nodes >> 7])

    g_idx_all = srcidx_of_node[row]
    is_lo_all = g_idx_all < cfg.RS2

    # per (core, slot): lo/hi edge arrays
    per_cs = {}
    klo_cs = np.zeros((NC, TS), dtype=np.int64)
    khi_cs = np.zeros((NC, TS), dtype=np.int64)
    for c in range(NC):
        for s in range(TS):
            t = assign[c, s]
            if t < 0 or t >= cfg.TILES_G or cnt[t] == 0:
                per_cs[(c, s)] = (np.zeros(0, np.int64), np.zeros(0, np.float32),
                                  np.zeros(0, np.float32),
                                  np.zeros(0, np.int64), np.zeros(0, np.float32),
                                  np.zeros(0, np.float32))
                continue
            eids = order[starts[t]:starts[t + 1]]
            g = g_idx_all[eids]
            cl = colloc[eids]
            tr = trend[eids]
            lo = is_lo_all[eids]
            per_cs[(c, s)] = (g[lo], cl[lo], tr[lo],
                              g[~lo] - cfg.HI2, cl[~lo], tr[~lo])
            klo_cs[c, s] = -(-len(g[lo]) // 128)
            khi_cs[c, s] = -(-(len(g) - len(g[lo])) // 128)

    KLO = klo_cs.max(axis=0)
    KHI = khi_cs.max(axis=0)
    empty = (KLO + KHI) == 0
    KLO[empty] = 1  # one pad chunk so the slot's PSUM is written (= zeros)
    cfg.KLO, cfg.KHI = KLO, KHI
    cfg.KMAX = int(max(KLO.max(), KHI.max()))

    # pack slots into blocks with chunk cap
    blocks, cur, cur_n = [], [], 0
    for s in range(TS):
        k = int(KLO[s] + KHI[s])
        if cur and cur_n + k > cfg.CHUNK_CAP:
            blocks.append(cur)
            cur, cur_n = [], 0
        cur.append(s)
        cur_n += k
    if cur:
        blocks.append(cur)
    cfg.blocks = blocks

    NCHUNKS = int((KLO + KHI).sum())
    cfg.NCHUNKS = NCHUNKS

    # per-core input arrays
    embed_f = np.zeros((cfg.SRC_ROWS, D), dtype=np.float32)
    embed_f[srcidx_of_node] = np.asarray(embed, dtype=np.float32)

    iota_in = np.tile(np.arange(128, dtype=np.float32),
                      (128, cfg.KMAX)).reshape(128, cfg.KMAX * 128)

    in_maps = []
    for c in range(NC):
        glo_parts, ghi_parts = [], []
        colq = np.zeros((NCHUNKS, 128), dtype=np.float32)
        trq = np.zeros((NCHUNKS, 128), dtype=np.float32)
        q = 0
        for blk in blocks:
            for kind in (0, 1):  # 0 = lo, 1 = hi
                for s in blk:
                    gi, cl, tr = per_cs[(c, s)][kind * 3:kind * 3 + 3]
                    K = int((KLO if kind == 0 else KHI)[s])
                    n = len(gi)
                    pad = K * 128 - n
                    gfull = np.concatenate([gi, np.zeros(pad, np.int64)])
                    cfull = np.concatenate([cl, np.zeros(pad, np.float32)])
                    tfull = np.concatenate([tr, np.zeros(pad, np.float32)])
                    (glo_parts if kind == 0 else ghi_parts).append(
                        gfull.astype(np.int16))
                    colq[q:q + K] = cfull.reshape(K, 128)
                    trq[q:q + K] = tfull.reshape(K, 128)
                    q += K
        assert q == NCHUNKS
        glo_all = np.concatenate(glo_parts) if glo_parts else np.zeros(0, np.int16)
        ghi_all = np.concatenate(ghi_parts) if ghi_parts else np.zeros(0, np.int16)
        in_maps.append({
            "embed_perm": embed_f,
            "gidx_lo": _wrap16(glo_all),
            "gidx_hi": _wrap16(ghi_all) if len(ghi_all) else
                np.zeros((128, 1), np.int16),
            "colloc_in": colq.T.copy(),   # [128, NCHUNKS]
            "trend_in": trq.T.copy(),
            "iota_in": iota_in,
        })
    return in_maps


def build(cfg: Cfg, repeat=1):
    D, TS, NC = cfg.D, cfg.TSLOTS, cfg.NCORES
    KLO, KHI, blocks = cfg.KLO, cfg.KHI, cfg.blocks
    NCHUNKS, KMAX = cfg.NCHUNKS, cfg.KMAX
    HOPS = cfg.HOPS
    NLO_TOT = int(KLO.sum()) * 128
    NHI_TOT = max(int(KHI.sum()) * 128, 16)
    CBMAX = max(int(sum(KLO[s] + KHI[s] for s in blk)) for blk in blocks)

    nc = bacc.Bacc("TRN2", target_bir_lowering=False, debug=False,
                   num_devices=NC)

    embed_perm = nc.dram_tensor("embed_perm", [cfg.SRC_ROWS, D], F32,
                                kind="ExternalInput")
    gidx_lo = nc.dram_tensor("gidx_lo", [128, NLO_TOT // 16], I16,
                             kind="ExternalInput")
    gidx_hi = nc.dram_tensor("gidx_hi", [128, NHI_TOT // 16], I16,
                             kind="ExternalInput")
    colloc_in = nc.dram_tensor("colloc_in", [128, NCHUNKS], F32,
                               kind="ExternalInput")
    trend_in = nc.dram_tensor("trend_in", [128, NCHUNKS], F32,
                              kind="ExternalInput")
    iota_in = nc.dram_tensor("iota_in", [128, KMAX * 128], F32,
                             kind="ExternalInput")
    out3 = nc.dram_tensor("out3", [HOPS, 128, TS * D], F32,
                          kind="ExternalOutput")

    aggs = [embed_perm] + [
        nc.dram_tensor(f"agg{h}", [cfg.SRC_ROWS, D], F32, addr_space="Shared")
        for h in range(1, HOPS)
    ]
    cc_in = [nc.dram_tensor(f"ccin{h}", [128, TS * D], F32)
             for h in range(HOPS - 1)]
    rg = [list(range(NC))]

    with tile.TileContext(nc) as tc:
        with (
            tc.tile_pool(name="meta", bufs=1) as meta,
            tc.tile_pool(name="gath", bufs=2) as gpool,
            tc.tile_pool(name="scal", bufs=2) as spool,
            tc.tile_pool(name="bmat", bufs=3) as bpool,
            tc.tile_pool(name="psum", bufs=4, space="PSUM") as ppool,
            tc.tile_pool(name="aggp", bufs=1) as apool,
        ):
            glo_sb = meta.tile([128, NLO_TOT // 16], I16)
            nc.sync.dma_start(glo_sb[:], gidx_lo[:])
            ghi_sb = meta.tile([128, NHI_TOT // 16], I16)
            nc.sync.dma_start(ghi_sb[:], gidx_hi[:])
            colloc_sb = meta.tile([128, NCHUNKS, 1], F32)
            nc.sync.dma_start(colloc_sb[:],
                              colloc_in[:].rearrange("p (a b) -> p a b", b=1))
            trend_sb = meta.tile([128, NCHUNKS, 1], F32)
            nc.sync.dma_start(trend_sb[:],
                              trend_in[:].rearrange("p (a b) -> p a b", b=1))
            iota_sb = meta.tile([128, KMAX, 128], F32)
            nc.sync.dma_start(iota_sb[:],
                              iota_in[:].rearrange("p (a b) -> p a b", b=128))

            agg_sb = apool.tile([128, TS * D], F32)

            for _rep in range(repeat):
              for h in range(HOPS):
                src = aggs[h].ap()
                lo_view = src[0:cfg.RS2, :]
                hi_view = src[cfg.HI2:cfg.SRC_ROWS, :]
                lo_off = 0   # position offset into gidx_lo stream
                hi_off = 0
                q0 = 0       # global chunk index
                for blk in blocks:
                    CLO_b = int(sum(KLO[s] for s in blk))
                    CHI_b = int(sum(KHI[s] for s in blk))
                    CB_b = CLO_b + CHI_b
                    NLO_b, NHI_b = CLO_b * 128, CHI_b * 128
                    gt = gpool.tile([128, CBMAX, D], F32, tag="gt")
                    nc.gpsimd.dma_gather(
                        gt[:, 0:CLO_b, :], lo_view,
                        glo_sb[:, lo_off // 16:(lo_off + NLO_b) // 16],
                        NLO_b, NLO_b, D, single_packet=False)
                    if CHI_b:
                        nc.gpsimd.dma_gather(
                            gt[:, CLO_b:CB_b, :], hi_view,
                            ghi_sb[:, hi_off // 16:(hi_off + NHI_b) // 16],
                            NHI_b, NHI_b, D, single_packet=False)
                    st = spool.tile([128, CBMAX, D], F32, tag="st")
                    nc.vector.tensor_tensor(
                        st[:, 0:CB_b, :], gt[:, 0:CB_b, :],
                        trend_sb[:, q0:q0 + CB_b, :].broadcast_to(
                            [128, CB_b, D]),
                        mybir.AluOpType.mult)
                    # chunk index of slot s's runs within this block
                    lo_base = q0
                    hi_base = q0 + CLO_b
                    lo_c = lo_base
                    hi_c = hi_base
                    for s in blk:
                        klo, khi = int(KLO[s]), int(KHI[s])
                        ps = ppool.tile([128, D], F32, tag="ps")
                        total_k = klo + khi
                        m = 0
                        for run_base, k in ((lo_c, klo), (hi_c, khi)):
                            if k == 0:
                                continue
                            bm = bpool.tile([128, KMAX, 128], F32, tag="bm")
                            nc.vector.tensor_tensor(
                                bm[:, 0:k, :], iota_sb[:, 0:k, :],
                                colloc_sb[:, run_base:run_base + k, :]
                                .broadcast_to([128, k, 128]),
                                mybir.AluOpType.is_equal)
                            for j in range(k):
                                nc.tensor.matmul(
                                    ps[:], bm[:, j, :],
                                    st[:, run_base - q0 + j, :],
                                    start=(m == 0),
                                    stop=(m == total_k - 1))
                                m += 1
                        lo_c += klo
                        hi_c += khi
                        nc.scalar.copy(agg_sb[:, s * D:(s + 1) * D], ps[:])
                    lo_off += NLO_b
                    hi_off += NHI_b
                    q0 += CB_b
                if h < HOPS - 1:
                    nc.sync.dma_start(cc_in[h].ap(), agg_sb[:])
                    nc.gpsimd.collective_compute(
                        "AllGather", mybir.AluOpType.bypass,
                        replica_groups=rg,
                        ins=[cc_in[h].ap().opt()],
                        outs=[aggs[h + 1].ap().opt()],
                    )
                nc.sync.dma_start(out3.ap()[h], agg_sb[:])
    nc.compile()
    return nc


def assemble(embed, results, cfg: Cfg):
    N, D, HOPS, TS = cfg.N, cfg.D, cfg.HOPS, cfg.TSLOTS
    out = np.empty((N, HOPS + 1, D), dtype=np.float32)
    out[:, 0, :] = np.asarray(embed, dtype=np.float32)
    for c in range(cfg.NCORES):
        o3 = np.asarray(results[c]["out3"]).reshape(HOPS, 128, TS, D)
        for s in range(TS):
            t = cfg.assign[c, s]
            if t < 0 or t >= cfg.TILES_G:
                continue
            base = t * 128
            pmax = min(128, N - base)
            for h in range(HOPS):
                out[base:base + pmax, h + 1, :] = o3[h, :pmax, s, :]
    return out


def run(embed, edge_index, trend, trace=False, trace_kwargs=None):
    cfg = Cfg()
    in_maps = preprocess(embed, edge_index, trend, cfg)
    nc = build(cfg)
    r = run_bass_kernel_spmd(
        nc, in_maps, core_ids=list(range(cfg.NCORES)),
        trace=trace, **(trace_kwargs or {}))
    return assemble(embed, r.results, cfg), r


def kernel(embed, edge_index, trend):
    out, _ = run(embed, edge_index, trend)
    return out


# revision 12
# speedup vs baseline: 21.6867x; 21.6867x over previous
"""GNN message passing (GraphConv_CA) kernel for Trainium2 (8 NeuronCores).

Problem: embed [50000, 64] f32; edge_index [2, 800000] i64; trend [800000] f32.
Per hop (x3): msg = agg[row] * trend; agg = segment_sum(msg, col, N).
Output: [50000, 4, 64] = concat(embed, hop1, hop2, hop3) along axis 1.

Strategy (gather + round-partitioned scatter-add):
  - Col-sharding: core c owns destination nodes [c*6250, (c+1)*6250). Edges
    are assigned to cores by col block, so every scatter-add is local to
    the owning core; an AllGather of the per-core [6250, 64] partials
    rebuilds the replicated [N, 64] agg for the next hop's gathers.
  - Per hop per core: dma_gather source rows from HBM (two instructions
    per block since gather indices are int16: rows >= 32768 use a view
    shifted by N-32768), one DVE multiply by per-edge trend per block,
    then dma_scatter_add into the local partial.
  - dma_scatter_add's CCE read-modify-write RACES on duplicate destination
    indices within one instruction (verified on HW, even for duplicates on
    one engine). So edges are partitioned into ROUNDS: round r holds the
    r-th edge of every destination col -> all indices within a round are
    unique. One scatter-add instruction per (round, lo/hi); instructions
    targeting the same buffer are serialized by the Tile framework's
    WAW dependencies, which wait for full DMA completion.
  - Padding positions gather row 0 with trend 0 and scatter 0.0 into
    per-position trash rows (cols 6250+) so every index is valid and all
    cores share one static program.
"""

import sys

sys.path.insert(0, "/opt/trn_rl_repo")

import numpy as np

import concourse.bacc as bacc
import concourse.mybir as mybir
import concourse.tile as tile
from concourse.bass_utils import run_bass_kernel_spmd

F32 = mybir.dt.float32
I16 = mybir.dt.int16


class Cfg:
    def __init__(self, N=50000, E=800000, D=64, HOPS=3, NCORES=8,
                 ROW_SPLIT=32768, POS_CAP=8000):
        assert N % NCORES == 0
        self.N, self.E, self.D, self.HOPS, self.NCORES = N, E, D, HOPS, NCORES
        self.NLOC = N // NCORES
        self.RS = min(ROW_SPLIT, N)
        self.HI_OFF = N - self.RS
        self.POS_CAP = POS_CAP        # max gathered positions per block
        self.DST_ROWS = self.NLOC + 150   # + trash rows for padding
        # filled by preprocess:
        self.rounds = None   # list of (S_lo, S_hi) padded sizes (x128)
        self.blocks = None   # list of lists of round ids


def _wrap16(arr):
    """[L] int16 -> [128, L//16]: position i at (i%16, i//16), replicated
    to 8 groups of 16 partitions."""
    w = arr.reshape(-1, 16).T
    return np.tile(w, (8, 1)).copy()


def preprocess(embed, edge_index, trend, cfg: Cfg):
    NC = cfg.NCORES
    row = np.asarray(edge_index[0], dtype=np.int64)
    col = np.asarray(edge_index[1], dtype=np.int64)
    trend = np.asarray(trend, dtype=np.float32)

    # per-core edges; round = rank of edge within its destination col
    per_core = []
    nrounds = 0
    for c in range(NC):
        m = (col // cfg.NLOC) == c
        r, cl, t = row[m], col[m] - c * cfg.NLOC, trend[m]
        o = np.argsort(cl, kind="stable")
        r, cl, t = r[o], cl[o], t[o]
        uniq, starts = np.unique(cl, return_index=True)
        rank = np.arange(len(cl)) - np.repeat(starts, np.diff(
            np.append(starts, len(cl))))
        lo = r < cfg.RS
        per_core.append((r, cl, t, rank, lo))
        if len(rank):
            nrounds = max(nrounds, int(rank.max()) + 1)

    # per (round, kind) padded sizes = max over cores, rounded to 128
    S = np.zeros((nrounds, 2), dtype=np.int64)
    for c in range(NC):
        r, cl, t, rank, lo = per_core[c]
        for kind in (0, 1):
            sel = rank[lo if kind == 0 else ~lo]
            if len(sel):
                cnts = np.bincount(sel, minlength=nrounds)
                S[:, kind] = np.maximum(S[:, kind], cnts)
    S = ((S + 127) // 128) * 128

    # blocks of consecutive rounds
    blocks, cur, cur_n = [], [], 0
    for ri in range(nrounds):
        k = int(S[ri].sum())
        if cur and cur_n + k > cfg.POS_CAP:
            blocks.append(cur)
            cur, cur_n = [], 0
        cur.append(ri)
        cur_n += k
    if cur:
        blocks.append(cur)
    cfg.rounds = S
    cfg.blocks = blocks
    NPOS = int(S.sum())
    cfg.NPOS = NPOS

    embed_f = np.ascontiguousarray(embed, dtype=np.float32)
    in_maps = []
    for c in range(NC):
        r, cl, t, rank, lo = per_core[c]
        glo = np.zeros(NPOS, np.int16)
        ghi = np.zeros(NPOS, np.int16)  # oversized; sliced per region
        scat = np.zeros(NPOS, np.int16)
        trd = np.zeros(NPOS, np.float32)
        # default scatter target: distinct trash rows per position-in-chunk
        scat[:] = (cfg.NLOC + (np.arange(NPOS) % 128)).astype(np.int16)

        glo_parts, ghi_parts = [], []
        pos = 0
        for blk in blocks:
            for kind in (0, 1):
                for ri in blk:
                    Sz = int(S[ri, kind])
                    if Sz == 0:
                        continue
                    sel = (lo if kind == 0 else ~lo) & (rank == ri)
                    rs, cs, ts = r[sel], cl[sel], t[sel]
                    n = len(rs)
                    idxs = np.zeros(Sz, np.int64)
                    idxs[:n] = rs - (0 if kind == 0 else cfg.HI_OFF)
                    gp = (glo_parts if kind == 0 else ghi_parts)
                    gp.append(idxs.astype(np.int16))
                    scat[pos:pos + n] = cs
                    trd[pos:pos + n] = ts
                    pos += Sz
        assert pos == NPOS
        glo_all = (np.concatenate(glo_parts) if glo_parts
                   else np.zeros(0, np.int16))
        ghi_all = (np.concatenate(ghi_parts) if ghi_parts
                   else np.zeros(0, np.int16))
        cfg.NLO_TOT = max(len(glo_all), 16)
        cfg.NHI_TOT = max(len(ghi_all), 16)
        in_maps.append({
            "embed": embed_f,
            "gidx_lo": _wrap16(np.resize(glo_all, cfg.NLO_TOT))
                if len(glo_all) else np.zeros((128, 1), np.int16),
            "gidx_hi": _wrap16(np.resize(ghi_all, cfg.NHI_TOT))
                if len(ghi_all) else np.zeros((128, 1), np.int16),
            "scat_idx": _wrap16(scat),
            "trend_in": trd.reshape(-1, 128).T.copy(),  # [128, NPOS//128]
        })
    return in_maps


def build(cfg: Cfg, repeat=1):
    D, NC, NLOC = cfg.D, cfg.NCORES, cfg.NLOC
    S, blocks = cfg.rounds, cfg.blocks
    HOPS = cfg.HOPS
    NPOS = cfg.NPOS
    NCH = NPOS // 128
    CBMAX = max(int(sum(S[ri].sum() for ri in blk)) for blk in blocks) // 128

    nc = bacc.Bacc("TRN2", target_bir_lowering=False, debug=False,
                   num_devices=NC)

    embed = nc.dram_tensor("embed", [cfg.N, D], F32, kind="ExternalInput")
    gidx_lo = nc.dram_tensor("gidx_lo", [128, max(cfg.NLO_TOT // 16, 1)], I16,
                             kind="ExternalInput")
    gidx_hi = nc.dram_tensor("gidx_hi", [128, max(cfg.NHI_TOT // 16, 1)], I16,
                             kind="ExternalInput")
    scat_idx = nc.dram_tensor("scat_idx", [128, NPOS // 16], I16,
                              kind="ExternalInput")
    trend_in = nc.dram_tensor("trend_in", [128, NCH], F32,
                              kind="ExternalInput")
    out3 = nc.dram_tensor("out3", [HOPS, cfg.DST_ROWS, D], F32,
                          kind="ExternalOutput")

    aggs = [embed] + [
        nc.dram_tensor(f"agg{h}", [cfg.N, D], F32, addr_space="Shared")
        for h in range(1, HOPS)
    ]
    cc_in = [nc.dram_tensor(f"ccin{h}", [NLOC, D], F32)
             for h in range(HOPS - 1)]
    rg = [list(range(NC))]

    with tile.TileContext(nc) as tc:
        with (
            tc.tile_pool(name="meta", bufs=1) as meta,
            tc.tile_pool(name="gath", bufs=2) as gpool,
            tc.tile_pool(name="scal", bufs=2) as spool,
        ):
            glo_sb = meta.tile([128, max(cfg.NLO_TOT // 16, 1)], I16)
            nc.sync.dma_start(glo_sb[:], gidx_lo[:])
            ghi_sb = meta.tile([128, max(cfg.NHI_TOT // 16, 1)], I16)
            nc.sync.dma_start(ghi_sb[:], gidx_hi[:])
            scat_sb = meta.tile([128, NPOS // 16], I16)
            nc.sync.dma_start(scat_sb[:], scat_idx[:])
            trend_sb = meta.tile([128, NCH, 1], F32)
            nc.sync.dma_start(trend_sb[:],
                              trend_in[:].rearrange("p (a b) -> p a b", b=1))

            for _rep in range(repeat):
              for h in range(HOPS):
                src = aggs[h].ap()
                lo_view = src[0:cfg.RS, :]
                hi_view = src[cfg.HI_OFF:cfg.N, :]
                dst = out3.ap()[h]
                lo_off = 0
                hi_off = 0
                q0 = 0
                for blk in blocks:
                    # per-block region sizes
                    NLO_b = int(sum(S[ri, 0] for ri in blk))
                    NHI_b = int(sum(S[ri, 1] for ri in blk))
                    CB_b = (NLO_b + NHI_b) // 128
                    CLO_b = NLO_b // 128
                    gt = gpool.tile([128, CBMAX, D], F32, tag="gt")
                    if NLO_b:
                        nc.gpsimd.dma_gather(
                            gt[:, 0:CLO_b, :], lo_view,
                            glo_sb[:, lo_off // 16:(lo_off + NLO_b) // 16],
                            NLO_b, NLO_b, D, single_packet=False)
                    if NHI_b:
                        nc.gpsimd.dma_gather(
                            gt[:, CLO_b:CB_b, :], hi_view,
                            ghi_sb[:, hi_off // 16:(hi_off + NHI_b) // 16],
                            NHI_b, NHI_b, D, single_packet=False)
                    st = spool.tile([128, CBMAX, D], F32, tag="st")
                    nc.vector.tensor_tensor(
                        st[:, 0:CB_b, :], gt[:, 0:CB_b, :],
                        trend_sb[:, q0:q0 + CB_b, :].broadcast_to(
                            [128, CB_b, D]),
                        mybir.AluOpType.mult)
                    # one scatter per (round, kind): unique dst cols
                    p = 0  # chunk offset within block
                    for kind in (0, 1):
                        for ri in blk:
                            Sz = int(S[ri, kind])
                            if Sz == 0:
                                continue
                            kc = Sz // 128
                            nc.gpsimd.dma_scatter_add(
                                dst, st[:, p:p + kc, :],
                                scat_sb[:, (q0 + p) * 8:(q0 + p + kc) * 8],
                                Sz, Sz, D, single_packet=False)
                            p += kc
                    lo_off += NLO_b
                    hi_off += NHI_b
                    q0 += CB_b
                if h < HOPS - 1:
                    nc.sync.dma_start(cc_in[h].ap(), dst[0:NLOC, :])
                    nc.gpsimd.collective_compute(
                        "AllGather", mybir.AluOpType.bypass,
                        replica_groups=rg,
                        ins=[cc_in[h].ap().opt()],
                        outs=[aggs[h + 1].ap().opt()],
                    )
    nc.compile()
    return nc


def assemble(embed, results, cfg: Cfg):
    N, D, HOPS = cfg.N, cfg.D, cfg.HOPS
    out = np.empty((N, HOPS + 1, D), dtype=np.float32)
    out[:, 0, :] = np.asarray(embed, dtype=np.float32)
    for c in range(cfg.NCORES):
        o3 = np.asarray(results[c]["out3"]).reshape(HOPS, cfg.DST_ROWS, D)
        sl = slice(c * cfg.NLOC, (c + 1) * cfg.NLOC)
        for h in range(HOPS):
            out[sl, h + 1, :] = o3[h, :cfg.NLOC, :]
    return out


def run(embed, edge_index, trend, trace=False, trace_kwargs=None):
    cfg = Cfg()
    in_maps = preprocess(embed, edge_index, trend, cfg)
    nc = build(cfg)
    r = run_bass_kernel_spmd(
        nc, in_maps, core_ids=list(range(cfg.NCORES)),
        trace=trace, **(trace_kwargs or {}))
    return assemble(embed, r.results, cfg), r


def kernel(embed, edge_index, trend):
    out, _ = run(embed, edge_index, trend)
    return out
